# revision 11
# baseline (speedup 1.0000x reference)
"""Trainium2 Bass kernel for ConvolutionalSelfAttention.

Math (per batch image):
  X [256, 64] pixels.  For each 3x3 window n (196 of them) and local slot k
  (9), the reference softmax-attends over the 247 pixels outside window n
  with logits TEMP*cos(x_g, x_{pix(n,k)}), weights s_g = x_g @ Wg + bg, and
  aggregates the window pixels with the resulting per-slot weights.

  Key factorization: all needed cosine sims live in one 256x256 gram
  E = exp(TEMP * Xn @ Xn.T); window/global masking is linear, so
      D[p, n] = sum_g maskg[g, n] * E[g, p]          (denominator)
      N[p, n] = sum_g maskg[g, n] * s'_g * E[g, p]   (numerator)
      A[p, n] = maskl[p, n] * N[p, n] / D[p, n]
      outT[c, n] = sum_p X[p, c] * A[p, n]
  -> everything is dense bf16 matmuls + one exp, no per-window gathers.

  E is symmetric, so the gram tiles e[chunk] = E[chunk pixels, all pixels]
  serve directly as the [contraction=g, rows=p] stationary operands.
  D and N share one 392-col matmul: the rhs is a 2-block strided AP
  [maskg | maskg*s'_b] over one per-half tile [mg | ml | ms_b0..b3].
  1/D runs as Ln -> Exp(-u) on the scalar engine (DVE reciprocal is
  7 cycles/elem); both functions live in one activation table.

Host does layout/prep only (~0.5% of FLOPs): casts to bf16, row-normalizes
X and ships it transposed (no device PE transposes), computes the tiny
per-pixel linear s' = x@Wg+bg and packs it into spare columns of the x
upload; all attention math (gram, exp, masked softmax matmuls,
aggregation) runs on device.

Sharding: data-parallel over batch; 32 images / 8 cores = 4 images per core.
"""

import sys
import numpy as np
import ml_dtypes

sys.path.insert(0, "/opt/trn_rl_repo")

from contextlib import ExitStack

import concourse.bass as bass
import concourse.bacc as bacc
import concourse.tile as tile
from concourse import mybir
from concourse.bass_utils import run_bass_kernel_spmd

H = 16
W = 16
C = 64
K = 3
B = 32
CH = H - K + 1
CW = W - K + 1
NC = CH * CW          # 196
HW = H * W            # 256
TEMP = 10.0
NCORES = 8
BPC = B // NCORES     # 4 images per core
P = 128
EPS = 1e-12

F32 = mybir.dt.float32
BF16 = mybir.dt.bfloat16
AF = mybir.ActivationFunctionType
BF = ml_dtypes.bfloat16


def _masks():
    maskl = np.zeros((HW, NC), np.float32)
    for i in range(CH):
        for j in range(CW):
            n = i * CW + j
            m = np.zeros((H, W), bool)
            m[i:i + K, j:j + K] = True
            maskl[m.reshape(-1), n] = 1.0
    return maskl, (1.0 - maskl).astype(np.float32)


MASKL, MASKG = _masks()
# fused [maskg | maskl] rows so the mask DMA moves 784B lines
MASKS_BF = np.ascontiguousarray(
    np.concatenate([MASKG, MASKL], axis=1).astype(BF))


def _patch_act_tables():
    """Steer every Ln/Exp activation to `natural_log_exp_and_others` so the
    kernel needs exactly one ACT table load instead of thrashing between the
    Ln-only and Exp-only sets (~2.7us per switch)."""
    from concourse import hw_specs
    orig_fn = hw_specs.get_activation_tables.__wrapped__

    def patched(arch):
        tabs = dict(orig_fn(arch))
        if "natural_log_exp_and_others" in tabs:
            for name in tabs:
                if name != "natural_log_exp_and_others":
                    tabs[name] = tabs[name] - {AF.Ln, AF.Exp}
        return tabs

    bacc.get_activation_tables = patched


def build_bass():
    _patch_act_tables()
    nc = bacc.Bacc("TRN2", target_bir_lowering=False, debug=False)

    # xb rows: [x(p,b0,:) .. x(p,b3,:), s'(p,b0..b3)] -> 260 bf16 = 520B lines
    xb = nc.declare_dram_parameter("xb", [HW, BPC * C + BPC], BF16, isOutput=False)
    xnt = nc.declare_dram_parameter("xnt", [C, BPC, HW], BF16, isOutput=False)
    mkd = nc.declare_dram_parameter("masks", [HW, 2 * NC], BF16, isOutput=False)
    y = nc.declare_dram_parameter("y", [C, BPC * NC], BF16, isOutput=True)

    with ExitStack() as ctx:
        tc = ctx.enter_context(tile.TileContext(nc))
        sb = ctx.enter_context(tc.tile_pool(name="sb", bufs=1))
        pg_pool = ctx.enter_context(tc.tile_pool(name="pg", bufs=2, space="PSUM"))
        pnd_pool = ctx.enter_context(tc.tile_pool(name="pnd", bufs=2, space="PSUM"))
        po_pool = ctx.enter_context(tc.tile_pool(name="po", bufs=2, space="PSUM"))

        # ---- inputs; xnT heads the critical path (sync queue), the rest
        # ---- rides the scalar engine's HWDGE queue in parallel.
        xnT = sb.tile([C, BPC * HW], BF16, tag="xnT")
        for h in range(2):
            nc.sync.dma_start(out=xnT[:, h * 2 * HW:(h + 1) * 2 * HW],
                              in_=xnt[:, 2 * h:2 * h + 2, :])

        # per-half working tile: [maskg | maskl | ms_b0 | ms_b1 | ms_b2 | ms_b3]
        M = []
        for t in range(2):
            mt = sb.tile([P, (2 + BPC) * NC], BF16, tag=f"M{t}")
            nc.scalar.dma_start(out=mt[:, :2 * NC], in_=mkd[t * P:(t + 1) * P, :])
            M.append(mt)

        xt = []
        for t in range(2):
            xtt = sb.tile([P, BPC * C + BPC], BF16, tag=f"x{t}")
            nc.scalar.dma_start(out=xtt, in_=xb[t * P:(t + 1) * P, :])
            xt.append(xtt)

        # ms_b = maskg * s'_b, written next to the masks
        spf = []
        for t in range(2):
            spt = sb.tile([P, BPC], F32, tag=f"sp{t}")
            nc.vector.tensor_copy(out=spt, in_=xt[t][:, BPC * C:])
            spf.append(spt)
        for t in range(2):
            for b in range(BPC):
                nc.vector.tensor_scalar_mul(
                    out=M[t][:, (2 + b) * NC:(3 + b) * NC],
                    in0=M[t][:, :NC],
                    scalar1=spf[t][:, b:b + 1])

        mlc = sb.tile([P, 2 * NC], BF16, tag="mlc")
        for pti in range(2):
            nc.gpsimd.tensor_copy(out=mlc[:, pti * NC:(pti + 1) * NC],
                                  in_=M[pti][:, NC:2 * NC])

        def dn_rhs(t, b):
            # 2-block strided view [maskg | ms_b] of M[t]
            mt = M[t]
            return bass.AP(tensor=mt.tensor, offset=mt.offset,
                           ap=[list(mt.ap[0]), [(2 + b) * NC, 2], [1, NC]])

        # ---- stage 2: gram + E = exp(TEMP * cos) ----
        e = []
        for b in range(BPC):
            g_ps = pg_pool.tile([P, 2 * HW], F32, tag="g")
            for chunk in range(2):
                nc.tensor.matmul(
                    out=g_ps[:, chunk * HW:(chunk + 1) * HW],
                    lhsT=xnT[:, b * HW + chunk * P: b * HW + (chunk + 1) * P],
                    rhs=xnT[:, b * HW:(b + 1) * HW],
                    start=True, stop=True)
            eb = sb.tile([P, 2 * HW], BF16, tag=f"e{b}")
            nc.scalar.activation(out=eb, in_=g_ps, func=AF.Exp, scale=TEMP)
            e.append(eb)

        # ---- stage 3: [D|N] matmuls; A = maskl * N / D ----
        # nd ring depth 4 keeps the PE ahead of the ln/exp/mul consumers;
        # u/rd/rdm are per-image [128, 392] ((pti, n) halves) to halve the
        # scalar/gpsimd instruction count.
        a = [sb.tile([P, BPC * NC], BF16, tag=f"a{pti}", name=f"a{pti}")
             for pti in range(2)]
        for b in range(BPC):
            nds = []
            u = sb.tile([P, 2 * NC], F32, tag=f"u{b}")
            for pti in range(2):
                nd = pnd_pool.tile([P, 2 * NC], F32, tag="nd")
                nc.tensor.matmul(out=nd, lhsT=e[b][:, pti * P:(pti + 1) * P],
                                 rhs=dn_rhs(0, b), start=True, stop=False)
                nc.tensor.matmul(out=nd,
                                 lhsT=e[b][:, HW + pti * P: HW + (pti + 1) * P],
                                 rhs=dn_rhs(1, b), start=False, stop=True)
                nc.scalar.activation(out=u[:, pti * NC:(pti + 1) * NC],
                                     in_=nd[:, :NC], func=AF.Ln)
                nds.append(nd)
            rd = sb.tile([P, 2 * NC], F32, tag=f"rd{b}")
            nc.scalar.activation(out=rd, in_=u, func=AF.Exp, scale=-1.0)
            rdm = sb.tile([P, 2 * NC], F32, tag=f"rdm{b}")
            nc.gpsimd.tensor_mul(out=rdm, in0=rd, in1=mlc)
            for pti in range(2):
                nc.vector.tensor_mul(out=a[pti][:, b * NC:(b + 1) * NC],
                                     in0=nds[pti][:, NC:],
                                     in1=rdm[:, pti * NC:(pti + 1) * NC])

        # ---- stage 4: outT = X.T @ A, image-paired matmuls ----
        # lhsT covers two images' channel blocks; out rows 0:64 belong to
        # the even image, 64:128 to the odd one.  Only the two diagonal
        # blocks of each [128, 392] psum are meaningful.
        yo = sb.tile([C, BPC * NC], BF16, tag="yo")
        for pr in range(BPC // 2):
            o_ps = po_pool.tile([P, 2 * NC], F32, tag="o")
            for pti in range(2):
                nc.tensor.matmul(
                    out=o_ps,
                    lhsT=xt[pti][:, pr * 2 * C:(pr + 1) * 2 * C],
                    rhs=a[pti][:, pr * 2 * NC:(pr + 1) * 2 * NC],
                    start=(pti == 0), stop=(pti == 1))
            nc.vector.tensor_copy(
                out=yo[:, 2 * pr * NC:(2 * pr + 1) * NC],
                in_=o_ps[:C, :NC])
            nc.vector.tensor_copy(
                out=yo[:, (2 * pr + 1) * NC:(2 * pr + 2) * NC],
                in_=o_ps[C:, NC:])
            nc.sync.dma_start(out=y[:, 2 * pr * NC:(2 * pr + 2) * NC],
                              in_=yo[:, 2 * pr * NC:(2 * pr + 2) * NC])

    nc.compile()
    return nc


_NC_CACHE = None


def _get_nc():
    global _NC_CACHE
    if _NC_CACHE is None:
        _NC_CACHE = build_bass()
    return _NC_CACHE


def make_in_maps(batch: np.ndarray, Wg: np.ndarray, bg: np.ndarray):
    X = np.asarray(batch, np.float32).reshape(B, HW, C)
    nrm = np.maximum(np.linalg.norm(X, axis=-1, keepdims=True), EPS)
    Xn = X / nrm
    sp = X @ np.asarray(Wg, np.float32).reshape(C) + np.asarray(bg, np.float32)
    # per-core layouts with contiguous DMA rows:
    #   xb  [HW, BPC*C + BPC]: (core, p, (b, c)) with s'(p, b) packed at the end
    #   xnt [C, BPC, HW]:      (core, c, b, p)
    xbm = X.reshape(NCORES, BPC, HW, C).transpose(0, 2, 1, 3).reshape(
        NCORES, HW, BPC * C)
    spm = sp.reshape(NCORES, BPC, HW).transpose(0, 2, 1)
    xb_bf = np.ascontiguousarray(
        np.concatenate([xbm, spm], axis=2).astype(BF))
    xnt_bf = np.ascontiguousarray(
        Xn.reshape(NCORES, BPC, HW, C).transpose(0, 3, 1, 2).astype(BF))
    return [
        {"xb": xb_bf[c], "xnt": xnt_bf[c], "masks": MASKS_BF}
        for c in range(NCORES)
    ]


def kernel(batch: np.ndarray, Wg: np.ndarray, bg: np.ndarray) -> np.ndarray:
    nc = _get_nc()
    in_maps = make_in_maps(batch, Wg, bg)
    res = run_bass_kernel_spmd(nc, in_maps, list(range(NCORES)))
    # y is [C, BPC*NC] bf16 per core -> [B, CH, CW, C] f32
    ys = np.stack([np.asarray(res.results[c]["y"]) for c in range(NCORES)], 0)
    out = ys.reshape(NCORES, C, BPC, NC).transpose(0, 2, 3, 1).astype(np.float32)
    return out.reshape(B, CH, CW, C)
